# revision 48
# baseline (speedup 1.0000x reference)
"""Trainium2 Bass kernel for nn_DenseConv2d_full (dense_cnn), fused v2.

Computation per sample b (8 samples, data-parallel over 8 NeuronCores):
  step 1: x[(ci,cr), y, w] = sum_{dy<16, dx<8} resp[cr,dy,dx] * imp[ci, y-dy, w-dx]
  step 2: y[oc, y, w] = bias[oc] + sum_{(ci,cr), my, mx} conv_w[oc,(ci,cr),my,mx]
          * x[(ci,cr), y+my-1, w+mx-1]   (3x3 SAME conv)

v2 design (vs the two-phase DRAM-roundtrip baseline, ~860us):
  * Fused single pipelined pass over 16-row blocks: phase A (dense conv via
    Toeplitz matmuls), a corner-turn of x into channel-partition layout, and
    phase B (3x3 merge conv) all overlap across blocks; B consumes x two
    blocks behind A for pipeline slack.
  * Impulse loads 17MB (vs baseline's 64MB host-materialized imp_big): one
    host-padded copy imp1[ci, H+16, W+8]; partition (s,u) holds row u
    pre-shifted by s for s in {0,1} (2x replication across the two PE
    row-tiles), dx-groups g in 0..3 are column offsets (dx = 7-2g-s).
  * Phase A per block: ci processed in pairs on the two 64-row PE tiles
    (K=64=(2s x 32u), M=128=(cr,yl), N=512, 4 accumulating matmuls per ci);
    the two row-tile chains stream concurrently, so PE wall time matches the
    K=128 form at half the impulse traffic. Evacs split DVE/ACT.
  * Corner turn ((cr,yl),(ci,x)) -> ((cr,ci),(yl,x)): impossible as one
    SBUF->SBUF DMA (partition-crossing strides are only legal in the
    outermost AP dim on both sides), so it bounces through DRAM. The
    scattered-write direction (1KB descriptor runs) is on the STORE side (8
    per-cr DMAs spread over 3 issuing engines) since nothing
    latency-critical waits on stores; the gather phase B depends on is one
    fully-contiguous 2MB load.
  * Phase B: groups of 4 row pairs, taps iterated outer so consecutive
    matmuls on a PE column tile share stationary weights (LDWEIGHTS
    amortized 4x); within a tap, (pair, half) matmuls interleave across the
    two column tiles, which execute concurrently (starts are pc-monotone,
    disjoint column tiles overlap) -> ~2x over the baseline's
    all-T0-then-all-T1 order. Edge taps mx=0/2 use N=511 with offset psum
    ranges so x tiles need no zero-guard columns (has_written semantics
    cover the partially-written psum columns). ScalarE does bias-add
    evacuation; one DMA stores each row pair.
  * Per-engine issue order keeps pair-critical ops (bias adds, y stores)
    ahead of the next block's turn DMAs on every sequencer queue (avoids
    priority inversion that exhausts psum pools and stalls the PE).
"""

import os
import sys
from contextlib import ExitStack

import numpy as np

for _p in (
    "/root/.axon_site",
    "/root/.axon_site/_ro/trn_rl_repo",
    "/root/.axon_site/_ro/pypackages",
    "/opt/trn_rl_repo",
):
    if os.path.isdir(_p) and _p not in sys.path:
        sys.path.append(_p)

import concourse.bass as bass  # noqa: E402
import concourse.tile as tile  # noqa: E402
from concourse import bacc, mybir  # noqa: E402
from concourse.bass_utils import run_bass_kernel_spmd  # noqa: E402

F32 = mybir.dt.float32
BF16 = mybir.dt.bfloat16

B, CR, KH, KW = 8, 8, 16, 8
CI, H, W = 16, 256, 512
OC = 64
NBLK = H // 16
WP = W + 2  # row segment width in x_blk (1 zero guard col each side)
IMPW = W + 8  # imp1 row length (8 left pad) and per-ci tile segment width

_BUILT = {}


def _build_nc(epochs=1, bench_mode=False, phases="AB"):
    nc = bacc.Bacc(
        "TRN2",
        target_bir_lowering=False,
        debug=False,
        enable_asserts=False,
        num_devices=8,
    )
    ikind = "Internal" if bench_mode else "ExternalInput"
    imp1 = nc.dram_tensor("imp1", [CI, H + 16, IMPW], BF16, kind=ikind).ap()
    w_toe = nc.dram_tensor("w_toe", [128, 4, 128], BF16, kind=ikind).ap()
    w9 = nc.dram_tensor("w9", [128, 9, OC], BF16, kind=ikind).ap()
    bias2 = nc.dram_tensor("bias2", [128, 1], F32, kind=ikind).ap()
    if bench_mode:
        nc.dram_tensor("dummy_in", [1, 1], F32, kind="ExternalInput")
    y_out = nc.dram_tensor(
        "y_out", [OC, H, W], F32, kind="Internal" if bench_mode else "ExternalOutput"
    ).ap()
    done = (
        nc.dram_tensor("done", [128, 1], F32, kind="ExternalOutput").ap()
        if bench_mode
        else None
    )
    xd = nc.dram_tensor("xd", [NBLK, 128, CI * W], BF16).ap()

    with tile.TileContext(nc) as tc, ExitStack() as ctx:
        consts = ctx.enter_context(tc.tile_pool(name="consts", bufs=1))
        imp_pool = ctx.enter_context(tc.tile_pool(name="imp", bufs=2))
        xb_pool = ctx.enter_context(tc.tile_pool(name="xev", bufs=3))
        xblk_pool = ctx.enter_context(tc.tile_pool(name="xblk", bufs=4))
        y2_pool = ctx.enter_context(tc.tile_pool(name="y2", bufs=6))
        psa_pool = ctx.enter_context(tc.tile_pool(name="psA", bufs=4, space="PSUM"))
        psb_pool = ctx.enter_context(tc.tile_pool(name="psB", bufs=4, space="PSUM"))

        wt = consts.tile([128, 4, 128], BF16)
        nc.sync.dma_start(wt[:], w_toe[:])
        w9t = consts.tile([128, 9, OC], BF16)
        nc.sync.dma_start(w9t[:], w9[:])
        bt = consts.tile([128, 1], F32)
        nc.sync.dma_start(bt[:], bias2[:])
        zrow = consts.tile([128, WP], BF16)
        nc.vector.memset(zrow[:], 0.0)

        env = dict(
            imp1=imp1, y_out=y_out, xd=xd, wt=wt, w9t=w9t, bt=bt, zrow=zrow,
            imp_pool=imp_pool, xb_pool=xb_pool, xblk_pool=xblk_pool,
            y2_pool=y2_pool, psa_pool=psa_pool, psb_pool=psb_pool,
        )
        for _ep in range(epochs):
            _epoch(nc, env, _ep, phases)
        if done is not None:
            nc.sync.dma_start(done, bt[:])

    nc.compile()
    return nc


def _epoch(nc, env, _ep, phases="AB"):
    imp1, y_out, xd = env["imp1"], env["y_out"], env["xd"]
    wt, w9t, bt, zrow = env["wt"], env["w9t"], env["bt"], env["zrow"]
    imp_pool, xb_pool, xblk_pool = env["imp_pool"], env["xb_pool"], env["xblk_pool"]
    y2_pool, psa_pool, psb_pool = env["y2_pool"], env["psa_pool"], env["psb_pool"]

    imp_tiles = {}
    xBs = {}
    ydma_engines = [nc.sync, nc.gpsimd]

    def load_imp(blk):
        """Load imp tile [64=(s,u), ci, IMPW] x2 row-tile replicas: partition
        (s,u) = imp1 row 16*blk+u pre-shifted left by s (s in {0,1}); the
        identical 64-partition block is replicated at partitions 64..127 for
        the second PE row-tile (2x replication instead of 4x)."""
        it = imp_pool.tile([128, CI, IMPW], BF16, tag="imp")
        for rep in range(2):
            for s in range(2):
                dst = bass.AP(
                    tensor=it.tensor,
                    offset=it.offset + (rep * 64 + s * 32) * (CI * IMPW),
                    ap=[[CI * IMPW, 32], [IMPW, CI], [1, W + 7]],
                )
                src = bass.AP(
                    tensor=imp1.tensor,
                    offset=imp1.offset + (16 * blk) * IMPW + s,
                    ap=[[IMPW, 32], [(H + 16) * IMPW, CI], [1, W + 7]],
                )
                (nc.gpsimd if (rep * 2 + s) % 2 else nc.sync).dma_start(dst, src)
        imp_tiles[blk] = it

    def a_mms(blk, it, xb):
        # ci processed in pairs on the two 64-row PE tiles (K=64, M=128):
        # row-tile rt handles ci=2*cp+rt; the two 4-matmul accumulation
        # chains stream concurrently (disjoint row groups).
        for cp in range(CI // 2):
            pss = []
            for rt in range(2):
                ci = 2 * cp + rt
                ps = psa_pool.tile(
                    [128, W], F32, tag="psa", name=f"psA_{_ep}_{blk}_{ci}"
                )
                pss.append(ps)
            for g in range(4):
                for rt in range(2):
                    ci = 2 * cp + rt
                    nc.tensor.matmul(
                        pss[rt][:, :],
                        lhsT=wt[64 * rt : 64 * rt + 64, g, :],
                        rhs=it[64 * rt : 64 * rt + 64, ci, 1 + 2 * g : 1 + 2 * g + W],
                        start=(g == 0),
                        stop=(g == 3),
                        tile_position=(64 * rt, 0),
                        skip_group_check=True,
                    )
            if xb is not None:
                for rt in range(2):
                    ci = 2 * cp + rt
                    # evacs split DVE/ACT: 16 on one engine would exceed the
                    # block's PE wall time and gate the psum pool
                    if ci % 2:
                        nc.scalar.copy(xb[:, ci * W : (ci + 1) * W], pss[rt][:])
                    else:
                        nc.vector.tensor_copy(
                            xb[:, ci * W : (ci + 1) * W], pss[rt][:]
                        )

    def a_block(blk):
        it = imp_tiles.pop(blk)
        xb = xb_pool.tile([128, CI * W], BF16, tag="xe")
        a_mms(blk, it, xb)
        return xb

    def a_turn(blk, xb):
        # corner-turn xb[(cr,yl),(ci,x)] -> xd[(cr,ci), (yl, x)] -> xB via
        # DRAM (SBUF DMA APs may only cross partitions in the outermost dim,
        # so a direct SBUF->SBUF repack is not expressible). The turn happens
        # on the STORE side (8 per-cr scattered-write DMAs, spread over 3
        # issuing engines): stores have pipeline slack and nothing
        # latency-critical waits on them. The gather that phase B depends on
        # is then a single fully-contiguous load. Issued AFTER the current
        # iteration's B pairs so pair-critical ops (bias adds, y stores) are
        # never queued behind the turn on any sequencer.
        xdb = xd[blk]
        store_engines = [nc.scalar, nc.sync, nc.gpsimd, nc.scalar, nc.gpsimd]
        for cr in range(8):
            src = bass.AP(
                tensor=xb.tensor,
                offset=xb.offset + cr * 16 * (CI * W),
                ap=[[CI * W, 16], [W, CI], [1, W]],
            )
            dst = bass.AP(
                tensor=xdb.tensor,
                offset=xdb.offset + cr * 16 * (CI * W),
                ap=[[W, 16], [CI * W, CI], [1, W]],
            )
            store_engines[cr % 5].dma_start(dst, src)
        xB = xblk_pool.tile([128, CI * W], BF16, tag="xblk")
        (nc.sync if blk % 2 else nc.gpsimd).dma_start(xB[:], xdb)
        xBs[blk] = xB
        for old in [k for k in xBs if k < blk - 2]:
            del xBs[old]

    def row_ref(y):
        if y < 0 or y >= H:
            return zrow, 0
        return xBs[y // 16], (y % 16) * W

    def b_group(Ys):
        # Taps iterate OUTER over a group of pairs: consecutive matmuls on a
        # column tile then share their stationary weights, amortizing
        # LDWEIGHTS 1:len(Ys) instead of 1:1 with the matmuls.
        pss = {}
        rtss = {}
        for Y in Ys:
            pss[Y] = psb_pool.tile([128, W], F32, tag="psb", name=f"psB_{_ep}_{Y}")
            rtss[Y] = [row_ref(Y - 1), row_ref(Y), row_ref(Y + 1), row_ref(Y + 2)]
        for t9 in range(9):
            my, mx = divmod(t9, 3)
            # mx=0/2 edge taps: N=511 with offset psum range, so no guard
            # columns are needed in the x tiles (out-of-range x[-1]/x[512]
            # contribute nothing; has_written handles the partially-covered
            # psum columns).
            c0 = 1 if mx == 0 else 0
            n = 512 if mx == 1 else 511
            roff = max(0, mx - 1)
            for Y in Ys:
                for half in range(2):
                    t, base = rtss[Y][my + half]
                    nc.tensor.matmul(
                        pss[Y][64 * half : 64 * half + 64, c0 : c0 + n],
                        lhsT=w9t[:, t9, :],
                        rhs=t[:, base + roff : base + roff + n],
                        start=(t9 == 0),
                        stop=(t9 == 8),
                        tile_position=(0, 64 * half),
                        skip_group_check=True,
                    )
        for Y in Ys:
            y2 = y2_pool.tile([128, W], F32, tag="y2")
            nc.scalar.add(y2[:], pss[Y][:], bt[:])
            ydma_engines[(Y // 2) % len(ydma_engines)].dma_start(
                y_out[:, Y : Y + 2, :].rearrange("oc h w -> h oc w"),
                y2[:],
            )

    def b_pair(Y):
        b_group([Y])

    def b_pairs_for(m):
        lo = 0 if m == 0 else 16 * m - 2
        ys = list(range(lo, 16 * m + 13, 2))
        for i in range(0, len(ys), 4):
            b_group(ys[i : i + 4])

    def a_mms_full(blk, it):
        # full-width M=128 variant (no tile_position): 2 MMs per ci
        for ci in range(CI):
            ps = psa_pool.tile([128, W], F32, tag="psa", name=f"psF_{_ep}_{blk}_{ci}")
            for g in range(2):
                nc.tensor.matmul(
                    ps[:, :],
                    lhsT=wt[:, 1 - g, :],
                    rhs=it[:, ci, 1 + 4 * g : 1 + 4 * g + W],
                    start=(g == 0),
                    stop=(g == 1),
                )

    if phases in ("M", "AE", "AS", "MF"):
        # matmul-stream isolation: imp loads + A matmuls
        # (+ evacs for AE; + evacs + store for AS)
        load_imp(0)
        load_imp(1)
        for blk in range(NBLK):
            if blk + 1 < NBLK and blk > 0:
                load_imp(blk + 1)
            it = imp_tiles.pop(blk)
            if phases == "MF":
                a_mms_full(blk, it)
                continue
            xb = None
            if phases in ("AE", "AS"):
                xb = xb_pool.tile([128, CI * W], BF16, tag="xe", name=f"xbe_{_ep}_{blk}")
            a_mms(blk, it, xb)
            if phases == "AS":
                nc.scalar.dma_start(xd[blk], xb[:])
        return
    if "B" not in phases:
        load_imp(0)
        load_imp(1)
        a_turn(0, a_block(0))
        for blk in range(1, NBLK):
            if blk + 1 < NBLK:
                load_imp(blk + 1)
            a_turn(blk, a_block(blk))
        return
    if "A" not in phases:
        # B only: static dummy x tiles (timing shape only, wrong data)
        xB0 = xblk_pool.tile([128, CI * W], BF16, tag="xblk", name=f"xB0_{_ep}")
        xBs[0] = xB0
        nc.vector.memset(xB0[:], 0.01)
        for blk in range(1, NBLK):
            xBs[blk] = xBs[0]
        for m in range(NBLK):
            b_pairs_for(m)
        b_pair(H - 2)
        return
    load_imp(0)
    load_imp(1)
    a_turn(0, a_block(0))
    for blk in range(1, NBLK):
        if blk + 1 < NBLK:
            load_imp(blk + 1)
        xb = a_block(blk)
        if blk >= 2:
            b_pairs_for(blk - 2)
        a_turn(blk, xb)
    b_pairs_for(NBLK - 2)
    b_pairs_for(NBLK - 1)
    b_pair(H - 2)


def _host_prep(response, impulse, conv_w, conv_b):
    """Per-sample input prep (pure layout + bf16 cast, no flops)."""
    import ml_dtypes

    bf16 = ml_dtypes.bfloat16
    in_maps = []
    # w9[(cr*16+ci), my*3+mx, oc] = conv_w[oc, ci*8+cr, my, mx]
    ch_perm = (np.arange(128) % 16) * 8 + np.arange(128) // 16
    w9 = np.ascontiguousarray(
        conv_w.transpose(1, 2, 3, 0).reshape(128, 9, OC)[ch_perm]
    ).astype(bf16)
    bias2 = np.tile(conv_b.astype(np.float32), 2).reshape(128, 1)
    # w_toe[(s,u), g, cr*16+yl] = resp[cr, yl+16-u, 7-2g-s], s in {0,1},
    # duplicated across the two 64-partition row-tile replicas.
    u_idx = np.arange(32)[:, None]
    yl_idx = np.arange(16)[None, :]
    dy = yl_idx + 16 - u_idx  # [32, 16]
    valid = (dy >= 0) & (dy < KH)
    for b in range(B):
        imp1 = np.zeros((CI, H + 16, IMPW), bf16)
        imp1[:, 16:, 8:] = impulse[b].astype(bf16)
        wt1 = np.zeros((2, 32, 4, 16, 8), np.float32)  # [s, u, g, yl, cr]
        resp = response[b]  # [cr, dy, dx]
        for s in range(2):
            for g in range(4):
                dx = 7 - 2 * g - s
                r = resp[:, np.clip(dy, 0, KH - 1), dx]  # [cr, 32, 16]
                wt1[s, :, g] = np.where(valid[None], r, 0.0).transpose(1, 2, 0)
        w64 = wt1.transpose(0, 1, 2, 4, 3).reshape(64, 4, 128)
        in_maps.append(
            {
                "imp1": imp1,
                "w_toe": np.concatenate([w64, w64], axis=0).astype(bf16),
                "w9": w9,
                "bias2": bias2,
            }
        )
    return in_maps


def kernel(response, impulse, conv_w, conv_b, _trace=False):
    response = np.asarray(response, np.float32)
    impulse = np.asarray(impulse, np.float32)
    conv_w = np.asarray(conv_w, np.float32)
    conv_b = np.asarray(conv_b, np.float32)

    if "nc" not in _BUILT:
        _BUILT["nc"] = _build_nc()
    nc = _BUILT["nc"]

    in_maps = _host_prep(response, impulse, conv_w, conv_b)
    res = run_bass_kernel_spmd(nc, in_maps, list(range(B)), trace=_trace)
    out = np.stack([res.results[b]["y_out"] for b in range(B)], axis=0)
    if _trace:
        _BUILT["last_exec_time_ns"] = res.exec_time_ns
        _BUILT["last_results"] = res
    return out


if __name__ == "__main__":
    data = np.load(os.path.join(os.path.dirname(__file__), "ref_cache.npz"))
    out = kernel(data["response"], data["impulse"], data["conv_w"], data["conv_b"])
    ref = data["out"]
    err = np.abs(out - ref).max() / np.abs(ref).max()
    print("Relative error:", err)


# revision 49
# speedup vs baseline: 1.1650x; 1.1650x over previous
"""Trainium2 Bass kernel for nn_DenseConv2d_full (dense_cnn), fused v2.

Computation per sample b (8 samples, data-parallel over 8 NeuronCores):
  step 1: x[(ci,cr), y, w] = sum_{dy<16, dx<8} resp[cr,dy,dx] * imp[ci, y-dy, w-dx]
  step 2: y[oc, y, w] = bias[oc] + sum_{(ci,cr), my, mx} conv_w[oc,(ci,cr),my,mx]
          * x[(ci,cr), y+my-1, w+mx-1]   (3x3 SAME conv)

v2 design (vs the two-phase DRAM-roundtrip baseline, ~860us):
  * Fused single pipelined pass over 16-row blocks: phase A (dense conv via
    Toeplitz matmuls), a corner-turn of x into channel-partition layout, and
    phase B (3x3 merge conv) all overlap across blocks; B consumes x two
    blocks behind A for pipeline slack.
  * Impulse loads 17MB (vs baseline's 64MB host-materialized imp_big): one
    host-padded copy imp1[ci, H+16, W+8]; partition (s,u) holds row u
    pre-shifted by s for s in {0,1} (2x replication across the two PE
    row-tiles), dx-groups g in 0..3 are column offsets (dx = 7-2g-s).
  * Phase A per block: ci processed in pairs on the two 64-row PE tiles
    (K=64=(2s x 32u), M=128=(cr,yl), N=512, 4 accumulating matmuls per ci);
    the two row-tile chains stream concurrently, so PE wall time matches the
    K=128 form at half the impulse traffic. Evacs split DVE/ACT.
  * Corner turn ((cr,yl),(ci,x)) -> ((cr,ci),(yl,x)): impossible as one
    SBUF->SBUF DMA (partition-crossing strides are only legal in the
    outermost AP dim on both sides), so it bounces through DRAM. The
    scattered-write direction (1KB descriptor runs) is on the STORE side (8
    per-cr DMAs spread over 3 issuing engines) since nothing
    latency-critical waits on stores; the gather phase B depends on is one
    fully-contiguous 2MB load.
  * Phase B: groups of 4 row pairs, taps iterated outer so consecutive
    matmuls on a PE column tile share stationary weights (LDWEIGHTS
    amortized 4x); within a tap, (pair, half) matmuls interleave across the
    two column tiles, which execute concurrently (starts are pc-monotone,
    disjoint column tiles overlap) -> ~2x over the baseline's
    all-T0-then-all-T1 order. Edge taps mx=0/2 use N=511 with offset psum
    ranges so x tiles need no zero-guard columns (has_written semantics
    cover the partially-written psum columns). ScalarE does bias-add
    evacuation; one DMA stores each row pair.
  * Per-engine issue order keeps pair-critical ops (bias adds, y stores)
    ahead of the next block's turn DMAs on every sequencer queue (avoids
    priority inversion that exhausts psum pools and stalls the PE).
"""

import os
import sys
from contextlib import ExitStack

import numpy as np

for _p in (
    "/root/.axon_site",
    "/root/.axon_site/_ro/trn_rl_repo",
    "/root/.axon_site/_ro/pypackages",
    "/opt/trn_rl_repo",
):
    if os.path.isdir(_p) and _p not in sys.path:
        sys.path.append(_p)

import concourse.bass as bass  # noqa: E402
import concourse.tile as tile  # noqa: E402
from concourse import bacc, mybir  # noqa: E402
from concourse.bass_utils import run_bass_kernel_spmd  # noqa: E402

F32 = mybir.dt.float32
BF16 = mybir.dt.bfloat16

B, CR, KH, KW = 8, 8, 16, 8
CI, H, W = 16, 256, 512
OC = 64
NBLK = H // 16
WP = W + 2  # row segment width in x_blk (1 zero guard col each side)
IMPW = W + 8  # imp1 row length (8 left pad) and per-ci tile segment width

_BUILT = {}


def _build_nc(epochs=1, bench_mode=False, phases="AB"):
    nc = bacc.Bacc(
        "TRN2",
        target_bir_lowering=False,
        debug=False,
        enable_asserts=False,
        num_devices=8,
    )
    ikind = "Internal" if bench_mode else "ExternalInput"
    imp1 = nc.dram_tensor("imp1", [CI, H + 16, IMPW], BF16, kind=ikind).ap()
    w_toe = nc.dram_tensor("w_toe", [128, 4, 128], BF16, kind=ikind).ap()
    w9 = nc.dram_tensor("w9", [128, 9, OC], BF16, kind=ikind).ap()
    bias2 = nc.dram_tensor("bias2", [128, 1], F32, kind=ikind).ap()
    if bench_mode:
        nc.dram_tensor("dummy_in", [1, 1], F32, kind="ExternalInput")
    y_out = nc.dram_tensor(
        "y_out", [OC, H, W], F32, kind="Internal" if bench_mode else "ExternalOutput"
    ).ap()
    done = (
        nc.dram_tensor("done", [128, 1], F32, kind="ExternalOutput").ap()
        if bench_mode
        else None
    )
    xd = nc.dram_tensor("xd", [NBLK, 128, CI * W], BF16).ap()

    with tile.TileContext(nc) as tc, ExitStack() as ctx:
        consts = ctx.enter_context(tc.tile_pool(name="consts", bufs=1))
        imp_pool = ctx.enter_context(tc.tile_pool(name="imp", bufs=2))
        xb_pool = ctx.enter_context(tc.tile_pool(name="xev", bufs=3))
        xblk_pool = ctx.enter_context(tc.tile_pool(name="xblk", bufs=4))
        y2_pool = ctx.enter_context(tc.tile_pool(name="y2", bufs=6))
        psa_pool = ctx.enter_context(tc.tile_pool(name="psA", bufs=3, space="PSUM"))
        psb_pool = ctx.enter_context(tc.tile_pool(name="psB", bufs=5, space="PSUM"))

        wt = consts.tile([128, 4, 128], BF16)
        nc.sync.dma_start(wt[:], w_toe[:])
        w9t = consts.tile([128, 9, OC], BF16)
        nc.sync.dma_start(w9t[:], w9[:])
        bt = consts.tile([128, 1], F32)
        nc.sync.dma_start(bt[:], bias2[:])
        zrow = consts.tile([128, WP], BF16)
        nc.vector.memset(zrow[:], 0.0)

        env = dict(
            imp1=imp1, y_out=y_out, xd=xd, wt=wt, w9t=w9t, bt=bt, zrow=zrow,
            imp_pool=imp_pool, xb_pool=xb_pool, xblk_pool=xblk_pool,
            y2_pool=y2_pool, psa_pool=psa_pool, psb_pool=psb_pool,
        )
        for _ep in range(epochs):
            _epoch(nc, env, _ep, phases)
        if done is not None:
            nc.sync.dma_start(done, bt[:])

    nc.compile()
    return nc


def _epoch(nc, env, _ep, phases="AB"):
    imp1, y_out, xd = env["imp1"], env["y_out"], env["xd"]
    wt, w9t, bt, zrow = env["wt"], env["w9t"], env["bt"], env["zrow"]
    imp_pool, xb_pool, xblk_pool = env["imp_pool"], env["xb_pool"], env["xblk_pool"]
    y2_pool, psa_pool, psb_pool = env["y2_pool"], env["psa_pool"], env["psb_pool"]

    imp_tiles = {}
    xBs = {}
    ydma_engines = [nc.sync, nc.gpsimd]

    def load_imp(blk):
        """Load imp tile [64=(s,u), ci, IMPW] x2 row-tile replicas: partition
        (s,u) = imp1 row 16*blk+u pre-shifted left by s (s in {0,1}); the
        identical 64-partition block is replicated at partitions 64..127 for
        the second PE row-tile (2x replication instead of 4x)."""
        it = imp_pool.tile([128, CI, IMPW], BF16, tag="imp")
        for rep in range(2):
            for s in range(2):
                dst = bass.AP(
                    tensor=it.tensor,
                    offset=it.offset + (rep * 64 + s * 32) * (CI * IMPW),
                    ap=[[CI * IMPW, 32], [IMPW, CI], [1, W + 7]],
                )
                src = bass.AP(
                    tensor=imp1.tensor,
                    offset=imp1.offset + (16 * blk) * IMPW + s,
                    ap=[[IMPW, 32], [(H + 16) * IMPW, CI], [1, W + 7]],
                )
                (nc.gpsimd if (rep * 2 + s) % 2 else nc.sync).dma_start(dst, src)
        imp_tiles[blk] = it

    def a_mms(blk, it, xb):
        # ci processed in pairs on the two 64-row PE tiles (K=64, M=128):
        # row-tile rt handles ci=2*cp+rt; the two 4-matmul accumulation
        # chains stream concurrently (disjoint row groups).
        for cp in range(CI // 2):
            pss = []
            for rt in range(2):
                ci = 2 * cp + rt
                ps = psa_pool.tile(
                    [128, W], F32, tag="psa", name=f"psA_{_ep}_{blk}_{ci}"
                )
                pss.append(ps)
            for g in range(4):
                for rt in range(2):
                    ci = 2 * cp + rt
                    nc.tensor.matmul(
                        pss[rt][:, :],
                        lhsT=wt[64 * rt : 64 * rt + 64, g, :],
                        rhs=it[64 * rt : 64 * rt + 64, ci, 1 + 2 * g : 1 + 2 * g + W],
                        start=(g == 0),
                        stop=(g == 3),
                        tile_position=(64 * rt, 0),
                        skip_group_check=True,
                    )
            if xb is not None:
                for rt in range(2):
                    ci = 2 * cp + rt
                    # evacs split DVE/ACT: 16 on one engine would exceed the
                    # block's PE wall time and gate the psum pool
                    if ci % 2:
                        nc.scalar.copy(xb[:, ci * W : (ci + 1) * W], pss[rt][:])
                    else:
                        nc.vector.tensor_copy(
                            xb[:, ci * W : (ci + 1) * W], pss[rt][:]
                        )

    def a_block(blk):
        it = imp_tiles.pop(blk)
        xb = xb_pool.tile([128, CI * W], BF16, tag="xe")
        a_mms(blk, it, xb)
        return xb

    def a_turn(blk, xb):
        # corner-turn xb[(cr,yl),(ci,x)] -> xd[(cr,ci), (yl, x)] -> xB via
        # DRAM (SBUF DMA APs may only cross partitions in the outermost dim,
        # so a direct SBUF->SBUF repack is not expressible). The turn happens
        # on the STORE side (8 per-cr scattered-write DMAs, spread over 3
        # issuing engines): stores have pipeline slack and nothing
        # latency-critical waits on them. The gather that phase B depends on
        # is then a single fully-contiguous load. Issued AFTER the current
        # iteration's B pairs so pair-critical ops (bias adds, y stores) are
        # never queued behind the turn on any sequencer.
        xdb = xd[blk]
        store_engines = [nc.scalar, nc.sync, nc.gpsimd, nc.scalar, nc.gpsimd]
        for cr in range(8):
            src = bass.AP(
                tensor=xb.tensor,
                offset=xb.offset + cr * 16 * (CI * W),
                ap=[[CI * W, 16], [W, CI], [1, W]],
            )
            dst = bass.AP(
                tensor=xdb.tensor,
                offset=xdb.offset + cr * 16 * (CI * W),
                ap=[[W, 16], [CI * W, CI], [1, W]],
            )
            store_engines[cr % 5].dma_start(dst, src)
        xB = xblk_pool.tile([128, CI * W], BF16, tag="xblk")
        (nc.sync if blk % 2 else nc.gpsimd).dma_start(xB[:], xdb)
        xBs[blk] = xB
        for old in [k for k in xBs if k < blk - 2]:
            del xBs[old]

    def row_ref(y):
        if y < 0 or y >= H:
            return zrow, 0
        return xBs[y // 16], (y % 16) * W

    def b_group(Ys):
        # Taps iterate OUTER over a group of pairs: consecutive matmuls on a
        # column tile then share their stationary weights, amortizing
        # LDWEIGHTS 1:len(Ys) instead of 1:1 with the matmuls.
        pss = {}
        rtss = {}
        for Y in Ys:
            pss[Y] = psb_pool.tile([128, W], F32, tag="psb", name=f"psB_{_ep}_{Y}")
            rtss[Y] = [row_ref(Y - 1), row_ref(Y), row_ref(Y + 1), row_ref(Y + 2)]
        for t9 in range(9):
            my, mx = divmod(t9, 3)
            # mx=0/2 edge taps: N=511 with offset psum range, so no guard
            # columns are needed in the x tiles (out-of-range x[-1]/x[512]
            # contribute nothing; has_written handles the partially-covered
            # psum columns).
            c0 = 1 if mx == 0 else 0
            n = 512 if mx == 1 else 511
            roff = max(0, mx - 1)
            for Y in Ys:
                for half in range(2):
                    t, base = rtss[Y][my + half]
                    nc.tensor.matmul(
                        pss[Y][64 * half : 64 * half + 64, c0 : c0 + n],
                        lhsT=w9t[:, t9, :],
                        rhs=t[:, base + roff : base + roff + n],
                        start=(t9 == 0),
                        stop=(t9 == 8),
                        tile_position=(0, 64 * half),
                        skip_group_check=True,
                    )
        for Y in Ys:
            y2 = y2_pool.tile([128, W], F32, tag="y2")
            nc.scalar.add(y2[:], pss[Y][:], bt[:])
            ydma_engines[(Y // 2) % len(ydma_engines)].dma_start(
                y_out[:, Y : Y + 2, :].rearrange("oc h w -> h oc w"),
                y2[:],
            )

    def b_pair(Y):
        b_group([Y])

    def b_pairs_for(m):
        lo = 0 if m == 0 else 16 * m - 2
        ys = list(range(lo, 16 * m + 13, 2))
        for i in range(0, len(ys), 4):
            b_group(ys[i : i + 4])

    def a_mms_full(blk, it):
        # full-width M=128 variant (no tile_position): 2 MMs per ci
        for ci in range(CI):
            ps = psa_pool.tile([128, W], F32, tag="psa", name=f"psF_{_ep}_{blk}_{ci}")
            for g in range(2):
                nc.tensor.matmul(
                    ps[:, :],
                    lhsT=wt[:, 1 - g, :],
                    rhs=it[:, ci, 1 + 4 * g : 1 + 4 * g + W],
                    start=(g == 0),
                    stop=(g == 1),
                )

    if phases in ("M", "AE", "AS", "MF"):
        # matmul-stream isolation: imp loads + A matmuls
        # (+ evacs for AE; + evacs + store for AS)
        load_imp(0)
        load_imp(1)
        for blk in range(NBLK):
            if blk + 1 < NBLK and blk > 0:
                load_imp(blk + 1)
            it = imp_tiles.pop(blk)
            if phases == "MF":
                a_mms_full(blk, it)
                continue
            xb = None
            if phases in ("AE", "AS"):
                xb = xb_pool.tile([128, CI * W], BF16, tag="xe", name=f"xbe_{_ep}_{blk}")
            a_mms(blk, it, xb)
            if phases == "AS":
                nc.scalar.dma_start(xd[blk], xb[:])
        return
    if "B" not in phases:
        load_imp(0)
        load_imp(1)
        a_turn(0, a_block(0))
        for blk in range(1, NBLK):
            if blk + 1 < NBLK:
                load_imp(blk + 1)
            a_turn(blk, a_block(blk))
        return
    if "A" not in phases:
        # B only: static dummy x tiles (timing shape only, wrong data)
        xB0 = xblk_pool.tile([128, CI * W], BF16, tag="xblk", name=f"xB0_{_ep}")
        xBs[0] = xB0
        nc.vector.memset(xB0[:], 0.01)
        for blk in range(1, NBLK):
            xBs[blk] = xBs[0]
        for m in range(NBLK):
            b_pairs_for(m)
        b_pair(H - 2)
        return
    load_imp(0)
    load_imp(1)
    a_turn(0, a_block(0))
    for blk in range(1, NBLK):
        if blk + 1 < NBLK:
            load_imp(blk + 1)
        xb = a_block(blk)
        if blk >= 2:
            b_pairs_for(blk - 2)
        a_turn(blk, xb)
    b_pairs_for(NBLK - 2)
    b_pairs_for(NBLK - 1)
    b_pair(H - 2)


def _host_prep(response, impulse, conv_w, conv_b):
    """Per-sample input prep (pure layout + bf16 cast, no flops)."""
    import ml_dtypes

    bf16 = ml_dtypes.bfloat16
    in_maps = []
    # w9[(cr*16+ci), my*3+mx, oc] = conv_w[oc, ci*8+cr, my, mx]
    ch_perm = (np.arange(128) % 16) * 8 + np.arange(128) // 16
    w9 = np.ascontiguousarray(
        conv_w.transpose(1, 2, 3, 0).reshape(128, 9, OC)[ch_perm]
    ).astype(bf16)
    bias2 = np.tile(conv_b.astype(np.float32), 2).reshape(128, 1)
    # w_toe[(s,u), g, cr*16+yl] = resp[cr, yl+16-u, 7-2g-s], s in {0,1},
    # duplicated across the two 64-partition row-tile replicas.
    u_idx = np.arange(32)[:, None]
    yl_idx = np.arange(16)[None, :]
    dy = yl_idx + 16 - u_idx  # [32, 16]
    valid = (dy >= 0) & (dy < KH)
    for b in range(B):
        imp1 = np.zeros((CI, H + 16, IMPW), bf16)
        imp1[:, 16:, 8:] = impulse[b].astype(bf16)
        wt1 = np.zeros((2, 32, 4, 16, 8), np.float32)  # [s, u, g, yl, cr]
        resp = response[b]  # [cr, dy, dx]
        for s in range(2):
            for g in range(4):
                dx = 7 - 2 * g - s
                r = resp[:, np.clip(dy, 0, KH - 1), dx]  # [cr, 32, 16]
                wt1[s, :, g] = np.where(valid[None], r, 0.0).transpose(1, 2, 0)
        w64 = wt1.transpose(0, 1, 2, 4, 3).reshape(64, 4, 128)
        in_maps.append(
            {
                "imp1": imp1,
                "w_toe": np.concatenate([w64, w64], axis=0).astype(bf16),
                "w9": w9,
                "bias2": bias2,
            }
        )
    return in_maps


def kernel(response, impulse, conv_w, conv_b, _trace=False):
    response = np.asarray(response, np.float32)
    impulse = np.asarray(impulse, np.float32)
    conv_w = np.asarray(conv_w, np.float32)
    conv_b = np.asarray(conv_b, np.float32)

    if "nc" not in _BUILT:
        _BUILT["nc"] = _build_nc()
    nc = _BUILT["nc"]

    in_maps = _host_prep(response, impulse, conv_w, conv_b)
    res = run_bass_kernel_spmd(nc, in_maps, list(range(B)), trace=_trace)
    out = np.stack([res.results[b]["y_out"] for b in range(B)], axis=0)
    if _trace:
        _BUILT["last_exec_time_ns"] = res.exec_time_ns
        _BUILT["last_results"] = res
    return out


if __name__ == "__main__":
    data = np.load(os.path.join(os.path.dirname(__file__), "ref_cache.npz"))
    out = kernel(data["response"], data["impulse"], data["conv_w"], data["conv_b"])
    ref = data["out"]
    err = np.abs(out - ref).max() / np.abs(ref).max()
    print("Relative error:", err)
